# revision 1
# baseline (speedup 1.0000x reference)
"""Ewald potential Bass kernels for TRN2 (8-core SPMD).

Strategy B: K1 shards k-space (480 cols/core of padded 3840) over all 8192
atoms -> akp=|k_pot| and v_pot (re/im). Host gathers. K2 shards atoms
(1024/core): aw GEMM -> softmax -> inverse transform, out.T per core.

Phases are computed in TURNS via bf16 GEMMs with 3-way-split rfrac
(exact products), range-reduced with the magic-number round on DVE,
then ACT Sin(scale=+-2pi).

out[n,d] = sum_k sm[n,k] * (cos(ph_i)*vpr[k,d] + sin(ph_i)*vpi[k,d]) / Z[n]
"""
import sys
sys.path.insert(0, '/opt/trn_rl_repo')
import numpy as np
import ml_dtypes
import concourse.bass as bass
import concourse.tile as tile
import concourse.mybir as mybir
from concourse import bacc
from concourse.bass_utils import run_bass_kernel_spmd
from concourse.dve_ops import ADD_RANGE_WRAP
from contextlib import ExitStack

F = mybir.ActivationFunctionType
DT = mybir.dt
ALU = mybir.AluOpType
AX = mybir.AxisListType

P = 128
N = 8192
D = 128
KPAD = 3840          # 3796 padded to 30*128 (pass-2 k chunks)
AWK = 4096           # aw/sm padded to 8*512 (PSUM bank alignment)
KSH = KPAD // 8      # 480 k-cols per core in K1
NSH = N // 8         # 1024 atoms per core in K2
NCH = N // P         # 64 atom chunks in K1
KCH = KPAD // P      # 30 k chunks in K2
NC2 = NSH // P       # 8 atom chunks in K2
MAGIC = 12582912.0   # 1.5 * 2^23
TWOPI = float(2 * np.pi)

bf16 = ml_dtypes.bfloat16


def split3(x):
    """3-way bf16 split of fp32 array: x ~ hi+mid+lo to ~1e-8."""
    hi = x.astype(bf16).astype(np.float32)
    r = x - hi
    mid = r.astype(bf16).astype(np.float32)
    lo = (r - mid).astype(bf16)
    return hi.astype(bf16), mid.astype(bf16), lo


def host_prep(q_vector, k_vector, v_vector, positions, cell, k_fwd, k_inv):
    L = float(np.asarray(cell).reshape(3, 3)[0, 0])
    rfrac = (np.asarray(positions, dtype=np.float32) / np.float32(L))  # [N,3]
    hi, mid, lo = split3(rfrac)
    rsplitT = np.concatenate([hi.T, mid.T, lo.T], axis=0)   # [9, N] bf16
    K = k_fwd.shape[0]

    def ktab9(kmat):  # [K,3] int -> [9, KPAD] bf16 (zero-padded)
        t = np.zeros((9, KPAD), dtype=np.float32)
        kT = kmat.T.astype(np.float32)
        t[0:3, :K] = kT
        t[3:6, :K] = kT
        t[6:9, :K] = kT
        return t.astype(bf16)

    ktabF = ktab9(np.asarray(k_fwd))
    ktabI = ktab9(np.asarray(k_inv))
    qT_abs = np.abs(np.asarray(q_vector, dtype=np.float32)).T.copy()  # [128,N]
    kv = np.ascontiguousarray(np.asarray(k_vector, dtype=np.float32))
    vv_bf = np.asarray(v_vector, dtype=np.float32).astype(bf16)
    return rsplitT, ktabF, ktabI, qT_abs, kv, vv_bf, K


# ---------------------------------------------------------------- kernel 1
def build_k1():
    nc = bacc.Bacc("TRN2", target_bir_lowering=False, debug=False)
    rsp_d = nc.dram_tensor("rsplitT", [9, N], DT.bfloat16, kind="ExternalInput").ap()
    ktab_d = nc.dram_tensor("ktab", [9, KSH], DT.bfloat16, kind="ExternalInput").ap()
    kv_d = nc.dram_tensor("kv", [N, D], DT.float32, kind="ExternalInput").ap()
    vv_d = nc.dram_tensor("vv", [N, D], DT.bfloat16, kind="ExternalInput").ap()
    akp_d = nc.dram_tensor("akp", [D, KSH], DT.float32, kind="ExternalOutput").ap()
    vpr_d = nc.dram_tensor("vpr", [D, KSH], DT.bfloat16, kind="ExternalOutput").ap()
    vpi_d = nc.dram_tensor("vpi", [D, KSH], DT.bfloat16, kind="ExternalOutput").ap()

    with ExitStack() as ctx:
        tc = ctx.enter_context(tile.TileContext(nc))
        cpool = ctx.enter_context(tc.tile_pool(name="const", bufs=1))
        wpool = ctx.enter_context(tc.tile_pool(name="work", bufs=3))
        pspool = ctx.enter_context(tc.tile_pool(name="ph", bufs=2, space="PSUM"))
        acc_ps = ctx.enter_context(tc.tile_pool(name="acc", bufs=1, space="PSUM"))

        rsp = cpool.tile([9, N], DT.bfloat16)
        ktab = cpool.tile([9, KSH], DT.bfloat16)
        kv = cpool.tile([P, NCH * D], DT.float32)    # [128 n-part, chunk-major d]
        vv = cpool.tile([P, NCH * D], DT.bfloat16)
        nc.sync.dma_start(rsp[:], rsp_d)
        nc.sync.dma_start(ktab[:], ktab_d)
        nc.sync.dma_start(kv[:].rearrange("p (c d) -> p c d", d=D),
                          kv_d.rearrange("(c p) d -> p c d", p=P))
        nc.sync.dma_start(vv[:].rearrange("p (c d) -> p c d", d=D),
                          vv_d.rearrange("(c p) d -> p c d", p=P))

        kre = acc_ps.tile([P, KSH], DT.float32)
        kim = acc_ps.tile([P, KSH], DT.float32)
        vre = acc_ps.tile([P, KSH], DT.float32)
        vim = acc_ps.tile([P, KSH], DT.float32)

        for c in range(NCH):
            ph = pspool.tile([P, KSH], DT.float32, tag="ph")
            nc.tensor.matmul(ph[:], rsp[:, c * P:(c + 1) * P], ktab[:],
                             start=True, stop=True)
            # t = round(phase) via magic number; PSUM -> SBUF
            tr = wpool.tile([P, KSH], DT.float32, tag="tr")
            nc.vector.tensor_scalar(tr[:], ph[:], MAGIC, MAGIC, ALU.add,
                                    ALU.subtract)
            # negr = (t + 0) - phase = -r
            negr = wpool.tile([P, KSH], DT.float32, tag="negr")
            nc.vector.scalar_tensor_tensor(negr[:], tr[:], 0.0, ph[:],
                                           ALU.add, ALU.subtract)
            # -rc = wrap(-r - 0.25); cos(2pi x) = sin(-2pi * -rc)
            negrc = wpool.tile([P, KSH], DT.float32, tag="negrc")
            nc.vector._custom_dve(ADD_RANGE_WRAP, out=negrc[:], in0=negr[:],
                                  s0=-0.25, s1=0.5, imm2=1.0)
            cosf = wpool.tile([P, KSH], DT.float32, tag="cosf")
            sinf = wpool.tile([P, KSH], DT.float32, tag="sinf")
            nc.scalar.activation(sinf[:], negr[:], F.Sin, scale=-TWOPI)
            nc.scalar.activation(cosf[:], negrc[:], F.Sin, scale=-TWOPI)
            cosb = wpool.tile([P, KSH], DT.bfloat16, tag="cosb")
            sinb = wpool.tile([P, KSH], DT.bfloat16, tag="sinb")
            nc.vector.tensor_copy(cosb[:], cosf[:])
            nc.vector.tensor_copy(sinb[:], sinf[:])
            st = dict(start=(c == 0), stop=(c == NCH - 1))
            nc.tensor.matmul(kre[:], kv[:, c * D:(c + 1) * D], cosf[:], **st)
            nc.tensor.matmul(kim[:], kv[:, c * D:(c + 1) * D], sinf[:], **st)
            nc.tensor.matmul(vre[:], vv[:, c * D:(c + 1) * D], cosb[:], **st)
            nc.tensor.matmul(vim[:], vv[:, c * D:(c + 1) * D], sinb[:], **st)

        # akp = sqrt(kre^2 + kim^2)
        sq1 = wpool.tile([P, KSH], DT.float32, tag="sq1")
        sq2 = wpool.tile([P, KSH], DT.float32, tag="sq2")
        nc.scalar.activation(sq1[:], kre[:], F.Square)
        nc.scalar.activation(sq2[:], kim[:], F.Square)
        ssum = wpool.tile([P, KSH], DT.float32, tag="ssum")
        nc.vector.tensor_add(ssum[:], sq1[:], sq2[:])
        akp = wpool.tile([P, KSH], DT.float32, tag="akp")
        nc.scalar.activation(akp[:], ssum[:], F.Sqrt)
        nc.sync.dma_start(akp_d, akp[:])
        vrb = wpool.tile([P, KSH], DT.bfloat16, tag="vrb")
        vib = wpool.tile([P, KSH], DT.bfloat16, tag="vib")
        nc.vector.tensor_copy(vrb[:], vre[:])
        nc.vector.tensor_copy(vib[:], vim[:])
        nc.sync.dma_start(vpr_d, vrb[:])
        nc.sync.dma_start(vpi_d, vib[:])

    nc.compile()
    return nc


# ---------------------------------------------------------------- kernel 2
def build_k2():
    nc = bacc.Bacc("TRN2", target_bir_lowering=False, debug=False)
    rsp_d = nc.dram_tensor("rsplitTloc", [9, NSH], DT.bfloat16,
                           kind="ExternalInput").ap()
    ktab_d = nc.dram_tensor("ktabI", [9, KPAD], DT.bfloat16,
                            kind="ExternalInput").ap()
    qT_d = nc.dram_tensor("qT", [D, NSH], DT.float32, kind="ExternalInput").ap()
    akp_d = nc.dram_tensor("akp", [D, AWK], DT.float32, kind="ExternalInput").ap()
    vprT_d = nc.dram_tensor("vprT", [KPAD, D], DT.bfloat16,
                            kind="ExternalInput").ap()
    vpiT_d = nc.dram_tensor("vpiT", [KPAD, D], DT.bfloat16,
                            kind="ExternalInput").ap()
    outT_d = nc.dram_tensor("outT", [D, NSH], DT.float32, kind="ExternalOutput").ap()
    zs_d = nc.dram_tensor("zs", [P, NC2], DT.float32, kind="ExternalOutput").ap()

    with ExitStack() as ctx:
        tc = ctx.enter_context(tile.TileContext(nc))
        cpool = ctx.enter_context(tc.tile_pool(name="const", bufs=1))
        smpool = ctx.enter_context(tc.tile_pool(name="sm", bufs=1))
        wpool = ctx.enter_context(tc.tile_pool(name="work", bufs=3))
        zpool = ctx.enter_context(tc.tile_pool(name="z", bufs=1))

        rsp = cpool.tile([9, NSH], DT.bfloat16)
        ktab = cpool.tile([9, KPAD], DT.bfloat16)
        qT = cpool.tile([D, NSH], DT.float32)
        akp = cpool.tile([D, AWK], DT.float32)
        vprT = cpool.tile([P, KCH * D], DT.bfloat16)  # [128 k-part, chunk-major d]
        vpiT = cpool.tile([P, KCH * D], DT.bfloat16)
        nc.sync.dma_start(rsp[:], rsp_d)
        nc.sync.dma_start(ktab[:], ktab_d)
        nc.sync.dma_start(qT[:], qT_d)
        nc.sync.dma_start(akp[:], akp_d)
        nc.sync.dma_start(vprT[:].rearrange("p (c d) -> p c d", d=D),
                          vprT_d.rearrange("(c p) d -> p c d", p=P))
        nc.sync.dma_start(vpiT[:].rearrange("p (c d) -> p c d", d=D),
                          vpiT_d.rearrange("(c p) d -> p c d", p=P))

        sm = smpool.tile([P, NC2 * AWK], DT.bfloat16)  # [128 n-part, chunk-major k]
        zacc = zpool.tile([P, NC2], DT.float32)

        # ---- pass 1: aw -> unnormalized softmax per atom chunk
        with tc.tile_pool(name="awps", bufs=1, space="PSUM") as awps:
            for c in range(NC2):
                aw = awps.tile([P, AWK], DT.float32, tag="aw")
                for j in range(16):
                    nc.tensor.matmul(aw[:, j * 256:(j + 1) * 256],
                                     qT[:, c * P:(c + 1) * P],
                                     akp[:, j * 256:(j + 1) * 256],
                                     start=True, stop=True)
                mx = wpool.tile([P, 1], DT.float32, tag="mx")
                nc.vector.reduce_max(mx[:], aw[:], axis=AX.X)
                negmx = wpool.tile([P, 1], DT.float32, tag="negmx")
                nc.vector.tensor_scalar_mul(negmx[:], mx[:], -1.0)
                nc.scalar.activation(sm[:, c * AWK:(c + 1) * AWK], aw[:], F.Exp,
                                     bias=negmx[:], accum_out=zacc[:, c:c + 1])

        # ---- pass 2: eik_i (transposed layout) + inverse transform
        with (tc.tile_pool(name="phps", bufs=2, space="PSUM") as phps,
              tc.tile_pool(name="ops", bufs=1, space="PSUM") as ops):
            outT = ops.tile([P, NSH], DT.float32)  # [128 d, 1024 n]
            for kc in range(KCH):
                ph = phps.tile([P, NSH], DT.float32, tag="ph")
                for h in range(2):
                    nc.tensor.matmul(ph[:, h * 512:(h + 1) * 512],
                                     ktab[:, kc * P:(kc + 1) * P],
                                     rsp[:, h * 512:(h + 1) * 512],
                                     start=True, stop=True)
                tr = wpool.tile([P, NSH], DT.float32, tag="tr")
                nc.vector.tensor_scalar(tr[:], ph[:], MAGIC, MAGIC, ALU.add,
                                        ALU.subtract)
                negr = wpool.tile([P, NSH], DT.float32, tag="negr")
                nc.vector.scalar_tensor_tensor(negr[:], tr[:], 0.0, ph[:],
                                               ALU.add, ALU.subtract)
                negrc = wpool.tile([P, NSH], DT.float32, tag="negrc")
                nc.vector._custom_dve(ADD_RANGE_WRAP, out=negrc[:], in0=negr[:],
                                      s0=-0.25, s1=0.5, imm2=1.0)
                cosi = wpool.tile([P, NSH], DT.bfloat16, tag="cosi")
                sini = wpool.tile([P, NSH], DT.bfloat16, tag="sini")
                nc.scalar.activation(sini[:], negr[:], F.Sin, scale=-TWOPI)
                nc.scalar.activation(cosi[:], negrc[:], F.Sin, scale=-TWOPI)
                # smT chunk [128 k, 1024 n] via 8 transposed DMAs
                smT = wpool.tile([P, NSH], DT.bfloat16, tag="smT")
                for c in range(NC2):
                    nc.sync.dma_start_transpose(
                        smT[:, c * P:(c + 1) * P],
                        sm[:, c * AWK + kc * P: c * AWK + (kc + 1) * P])
                smC = wpool.tile([P, NSH], DT.bfloat16, tag="smC")
                smS = wpool.tile([P, NSH], DT.bfloat16, tag="smS")
                nc.vector.tensor_mul(smC[:], smT[:], cosi[:])
                nc.vector.tensor_mul(smS[:], smT[:], sini[:])
                # out.T += vprT_c.T @ smC + vpiT_c.T @ smS
                for h in range(2):
                    hs = slice(h * 512, (h + 1) * 512)
                    nc.tensor.matmul(outT[:, hs], vprT[:, kc * D:(kc + 1) * D],
                                     smC[:, hs], start=(kc == 0), stop=False)
                    nc.tensor.matmul(outT[:, hs], vpiT[:, kc * D:(kc + 1) * D],
                                     smS[:, hs], start=False,
                                     stop=(kc == KCH - 1))

            res = wpool.tile([P, NSH], DT.float32, tag="res")
            nc.vector.tensor_copy(res[:], outT[:])
            nc.sync.dma_start(outT_d, res[:])
            nc.sync.dma_start(zs_d, zacc[:])

    nc.compile()
    return nc


# ---------------------------------------------------------------- profiling
def enable_ntff_profiling():
    """Provide the antenv.axon_hooks module run_bass_kernel_spmd needs for
    trace=True under axon, backed by trn_boot's ctypes NTFF hook."""
    import types
    if "antenv.axon_hooks" in sys.modules:
        return True
    sys.path.insert(0, "/root/.axon_site")
    try:
        from trn_agent_boot.trn_boot import _ntff_profile_via_ctypes
        hook = _ntff_profile_via_ctypes("/opt/axon/libaxon_pjrt.so")
    except Exception as e:
        print(f"ntff hook unavailable: {e}")
        return False
    if hook is None:
        print("ntff hook: .so lacks axon_start_nrt_profile")
        return False
    mod = types.ModuleType("antenv.axon_hooks")
    mod._hook = hook
    mod.get_axon_ntff_profile_hook = lambda: mod._hook
    mod.set_axon_ntff_profile_hook = lambda h: setattr(mod, "_hook", h)
    sys.modules["antenv.axon_hooks"] = mod
    # upload_artifacts copies the NEFF dir to a remote bucket -- hangs in
    # this container; keep artifacts local instead.
    import concourse.bass_utils as bu
    bu.upload_artifacts = lambda tmpdir: tmpdir
    return True


# ---------------------------------------------------------------- runner
_NC1 = None
_NC2 = None


def run_ewald(q_vector, k_vector, v_vector, positions, cell, batch, k_fwd,
              k_inv, trace=False):
    global _NC1, _NC2
    if trace:
        trace = enable_ntff_profiling()
    rsplitT, ktabF, ktabI, qT_abs, kv, vv_bf, K = host_prep(
        q_vector, k_vector, v_vector, positions, cell, k_fwd, k_inv)

    if _NC1 is None:
        _NC1 = build_k1()
    in1 = [{"rsplitT": np.ascontiguousarray(rsplitT),
            "ktab": np.ascontiguousarray(ktabF[:, c * KSH:(c + 1) * KSH]),
            "kv": kv, "vv": vv_bf} for c in range(8)]
    r1 = run_bass_kernel_spmd(_NC1, in1, list(range(8)), trace=trace)

    akp = np.concatenate([r1.results[c]["akp"] for c in range(8)], axis=1)
    vpr = np.concatenate([r1.results[c]["vpr"] for c in range(8)], axis=1)
    vpi = np.concatenate([r1.results[c]["vpi"] for c in range(8)], axis=1)
    akp[:, K:] = 0.0
    akp_pad = np.zeros((D, AWK), dtype=np.float32)
    akp_pad[:, :KPAD] = akp
    vprT = np.ascontiguousarray(vpr.T)  # [KPAD, 128] bf16
    vpiT = np.ascontiguousarray(vpi.T)
    vprT[K:, :] = 0
    vpiT[K:, :] = 0

    if _NC2 is None:
        _NC2 = build_k2()
    in2 = [{"rsplitTloc": np.ascontiguousarray(rsplitT[:, c * NSH:(c + 1) * NSH]),
            "ktabI": np.ascontiguousarray(ktabI),
            "qT": np.ascontiguousarray(qT_abs[:, c * NSH:(c + 1) * NSH]),
            "akp": akp_pad, "vprT": vprT, "vpiT": vpiT} for c in range(8)]
    r2 = run_bass_kernel_spmd(_NC2, in2, list(range(8)), trace=trace)

    outs = []
    for c in range(8):
        oT = r2.results[c]["outT"]               # [128 d, 1024 n]
        z = r2.results[c]["zs"].T.reshape(-1)    # atom n=c*128+p -> zs[p, c]
        outs.append((oT.T / z[:, None]).astype(np.float32))
    out = np.concatenate(outs, axis=0)
    return out, (r1, r2)


# ---------------------------------------------------------------- entry point
def kernel(q_vector, k_vector, v_vector, positions, cell, batch, k_fwd, k_inv):
    """Full-input entry: shards across 8 NeuronCores internally."""
    out, _ = run_ewald(np.asarray(q_vector), np.asarray(k_vector),
                       np.asarray(v_vector), np.asarray(positions),
                       np.asarray(cell), np.asarray(batch),
                       np.asarray(k_fwd), np.asarray(k_inv))
    return out



# revision 6
# speedup vs baseline: 1.6831x; 1.6831x over previous
"""Ewald potential Bass kernels for TRN2 (8-core SPMD), v2.

K1 shards k-space (480 cols/core of padded 3840) over all 8192 atoms ->
k_pot re/im (fp32) and v_pot re/im (fp16). Host gathers, computes
akp=|k_pot| and fp16 splits. K2 shards atoms (1024/core): aw GEMM (3-term
fp16 split) -> softmax -> inverse transform via PE-transposed sm.

Phases come from a one-hot selection GEMM against host-precomputed
frac(coord*k) tables centered in [-0.5,0.5]: phase' = Tx+Ty+Tz in
[-1.5,1.5], range-reduced with a single ADD_RANGE_WRAP, cos via a second
wrap (+0.25). Sin activation with scale=2pi.

out[n,d] = sum_k sm[n,k] * (cos(2pi phi_i)*vpr[k,d] + sin(2pi phi_i)*vpi[k,d]) / Z[n]
with eik_i = exp(-2pi i phi_i) = cos - i sin.
"""
import sys
sys.path.insert(0, '/opt/trn_rl_repo')
import numpy as np
import ml_dtypes
import concourse.bass as bass
import concourse.tile as tile
import concourse.mybir as mybir
from concourse import bacc
from concourse.bass_utils import run_bass_kernel_spmd
from contextlib import ExitStack

F = mybir.ActivationFunctionType
DT = mybir.dt
ALU = mybir.AluOpType
AX = mybir.AxisListType

P = 128
N = 8192
D = 128
NK = 12              # grid: kx in [0,12], ky/kz in [-12,12]
KPAD = 3840          # 3796 padded to 30*128
KSH = KPAD // 8      # 480 k-cols per core in K1
NSH = N // 8         # 1024 atoms per core in K2
NCH = N // P         # 64 atom chunks in K1
KCH = KPAD // P      # 30 k chunks in K2
AWK = 4096           # aw/sm width per n-chunk (2 halves of 2048)
NC2 = NSH // P       # 8 atom chunks in K2
NROW = 63            # 13 x-rows + 25 y-rows + 25 z-rows
TWOPI = float(2 * np.pi)

bf16 = ml_dtypes.bfloat16
f16 = np.float16


def _frac_tables(rfrac):
    """[63, n] fp64 tables: frac(coord*u) centered to [-0.5, 0.5]."""
    n = rfrac.shape[0]
    t = np.zeros((NROW, n), dtype=np.float64)
    r64 = rfrac.astype(np.float64)
    for u in range(NK + 1):                      # x rows: u = 0..12
        v = r64[:, 0] * u
        t[u] = v - np.round(v)
    for i, u in enumerate(range(-NK, NK + 1)):   # y rows
        v = r64[:, 1] * u
        t[13 + i] = v - np.round(v)
    for i, u in enumerate(range(-NK, NK + 1)):   # z rows
        v = r64[:, 2] * u
        t[38 + i] = v - np.round(v)
    return t


def _select_mat(kmat):
    """[63, KPAD] fp16 one-hot selection for k rows (padded cols zero)."""
    K = kmat.shape[0]
    s = np.zeros((NROW, KPAD), dtype=np.float32)
    j = np.arange(K)
    s[kmat[:, 0], j] = 1.0
    s[13 + kmat[:, 1] + NK, j] = 1.0
    s[38 + kmat[:, 2] + NK, j] = 1.0
    return s.astype(f16)


def split16(x):
    """fp16 2-way split: x ~ hi + lo to ~2^-22 rel."""
    x = np.asarray(x, dtype=np.float32)
    hi = x.astype(f16)
    lo = (x - hi.astype(np.float32)).astype(f16)
    return hi, lo


def host_prep(q_vector, k_vector, v_vector, positions, cell, k_fwd, k_inv):
    L = float(np.asarray(cell).reshape(3, 3)[0, 0])
    rfrac = np.asarray(positions, dtype=np.float64) / L
    t64 = _frac_tables(rfrac)                     # [63, N]
    th = t64.astype(f16)
    tl = (t64 - th.astype(np.float64)).astype(f16)
    sf = _select_mat(np.asarray(k_fwd))           # [63, KPAD]
    si = _select_mat(np.asarray(k_inv))
    kvh, kvl = split16(k_vector)                  # [N, D]
    vvh = np.asarray(v_vector, dtype=np.float32).astype(f16)
    qh, ql = split16(np.abs(np.asarray(q_vector, dtype=np.float32)).T)  # [D, N]
    return th, tl, sf, si, kvh, kvl, vvh, qh, ql


def chunk_major(x):
    """[N, D] -> [P, NCH*D]: partition=n%P? No: chunk c rows c*P..(c+1)*P
    land at [:, c*D:(c+1)*D]."""
    n, d = x.shape
    c = n // P
    return np.ascontiguousarray(
        x.reshape(c, P, d).transpose(1, 0, 2).reshape(P, c * d))


# ---------------------------------------------------------------- kernel 1
def build_k1():
    nc = bacc.Bacc("TRN2", target_bir_lowering=False, debug=False)
    th_d = nc.dram_tensor("th", [NROW, N], DT.float16, kind="ExternalInput").ap()
    tl_d = nc.dram_tensor("tl", [NROW, N], DT.float16, kind="ExternalInput").ap()
    sf_d = nc.dram_tensor("sf", [NROW, KSH], DT.float16, kind="ExternalInput").ap()
    kvh_d = nc.dram_tensor("kvh", [P, NCH * D], DT.float16, kind="ExternalInput").ap()
    kvl_d = nc.dram_tensor("kvl", [P, NCH * D], DT.float16, kind="ExternalInput").ap()
    vvh_d = nc.dram_tensor("vvh", [P, NCH * D], DT.float16, kind="ExternalInput").ap()
    kre_d = nc.dram_tensor("kre", [P, KSH], DT.float32, kind="ExternalOutput").ap()
    kim_d = nc.dram_tensor("kim", [P, KSH], DT.float32, kind="ExternalOutput").ap()
    vre_d = nc.dram_tensor("vre", [P, KSH], DT.float16, kind="ExternalOutput").ap()
    vim_d = nc.dram_tensor("vim", [P, KSH], DT.float16, kind="ExternalOutput").ap()

    with ExitStack() as ctx:
        tc = ctx.enter_context(tile.TileContext(nc))
        cpool = ctx.enter_context(tc.tile_pool(name="const", bufs=1))
        wpool = ctx.enter_context(tc.tile_pool(name="work", bufs=3))
        php = ctx.enter_context(tc.tile_pool(name="ph", bufs=2, space="PSUM"))
        accp = ctx.enter_context(tc.tile_pool(name="acc", bufs=1, space="PSUM"))

        th = cpool.tile([NROW, N], DT.float16)
        tlo = cpool.tile([NROW, N], DT.float16)
        sf = cpool.tile([NROW, KSH], DT.float16)
        kvh = cpool.tile([P, NCH * D], DT.float16)
        kvl = cpool.tile([P, NCH * D], DT.float16)
        vvh = cpool.tile([P, NCH * D], DT.float16)
        nc.sync.dma_start(th[:], th_d)
        nc.sync.dma_start(tlo[:], tl_d)
        nc.sync.dma_start(sf[:], sf_d)
        nc.sync.dma_start(kvh[:], kvh_d)
        nc.sync.dma_start(kvl[:], kvl_d)
        nc.sync.dma_start(vvh[:], vvh_d)

        kre = accp.tile([P, 512], DT.float32)
        kim = accp.tile([P, 512], DT.float32)
        vre = accp.tile([P, 512], DT.float32)
        vim = accp.tile([P, 512], DT.float32)

        for g in range(NCH // 2):
            ph = php.tile([P, 1024], DT.float32, tag="ph")
            for h in range(2):
                c = 2 * g + h
                sl = slice(h * 512, h * 512 + KSH)
                nc.tensor.matmul(ph[:, sl], th[:, c * P:(c + 1) * P], sf[:],
                                 start=True, stop=False)
                nc.tensor.matmul(ph[:, sl], tlo[:, c * P:(c + 1) * P], sf[:],
                                 start=False, stop=True)
            r = wpool.tile([P, 1024], DT.float32, tag="r")
            nc.vector.add_range_wrap(r[:], ph[:], 0.0, 0.5, 1.0)
            w2 = wpool.tile([P, 1024], DT.float32, tag="w2")
            nc.vector.add_range_wrap(w2[:], r[:], 0.25, 0.5, 1.0)
            sinf = wpool.tile([P, 1024], DT.float16, tag="sinf")
            cosf = wpool.tile([P, 1024], DT.float16, tag="cosf")
            nc.scalar.activation(sinf[:], r[:], F.Sin, scale=TWOPI)
            nc.scalar.activation(cosf[:], w2[:], F.Sin, scale=TWOPI)
            for h in range(2):
                c = 2 * g + h
                sl = slice(h * 512, h * 512 + KSH)
                dsl = slice(c * D, (c + 1) * D)
                st = dict(start=(c == 0), stop=False)
                en = dict(start=False, stop=(c == NCH - 1))
                nc.tensor.matmul(kre[:, :KSH], kvh[:, dsl], cosf[:, sl], **st)
                nc.tensor.matmul(kre[:, :KSH], kvl[:, dsl], cosf[:, sl], **en)
                nc.tensor.matmul(kim[:, :KSH], kvh[:, dsl], sinf[:, sl], **st)
                nc.tensor.matmul(kim[:, :KSH], kvl[:, dsl], sinf[:, sl], **en)
                nc.tensor.matmul(vre[:, :KSH], vvh[:, dsl], cosf[:, sl],
                                 start=(c == 0), stop=(c == NCH - 1))
                nc.tensor.matmul(vim[:, :KSH], vvh[:, dsl], sinf[:, sl],
                                 start=(c == 0), stop=(c == NCH - 1))

        kre_s = wpool.tile([P, KSH], DT.float32, tag="kre_s")
        kim_s = wpool.tile([P, KSH], DT.float32, tag="kim_s")
        vre_s = wpool.tile([P, KSH], DT.float16, tag="vre_s")
        vim_s = wpool.tile([P, KSH], DT.float16, tag="vim_s")
        nc.vector.tensor_copy(kre_s[:], kre[:, :KSH])
        nc.vector.tensor_copy(kim_s[:], kim[:, :KSH])
        nc.vector.tensor_copy(vre_s[:], vre[:, :KSH])
        nc.vector.tensor_copy(vim_s[:], vim[:, :KSH])
        nc.sync.dma_start(kre_d, kre_s[:])
        nc.sync.dma_start(kim_d, kim_s[:])
        nc.sync.dma_start(vre_d, vre_s[:])
        nc.sync.dma_start(vim_d, vim_s[:])

    nc.compile()
    return nc


# ---------------------------------------------------------------- kernel 2
def build_k2():
    nc = bacc.Bacc("TRN2", target_bir_lowering=False, debug=False)
    t2_d = nc.dram_tensor("t2", [NROW, NSH], DT.float16, kind="ExternalInput").ap()
    si_d = nc.dram_tensor("si", [NROW, KPAD], DT.float16, kind="ExternalInput").ap()
    qh_d = nc.dram_tensor("qh", [P, NSH], DT.float16, kind="ExternalInput").ap()
    ql_d = nc.dram_tensor("ql", [P, NSH], DT.float16, kind="ExternalInput").ap()
    ah_d = nc.dram_tensor("ah", [P, AWK], DT.float16, kind="ExternalInput").ap()
    al_d = nc.dram_tensor("al", [P, AWK], DT.float16, kind="ExternalInput").ap()
    vprT_d = nc.dram_tensor("vprT", [P, KCH * D], DT.bfloat16,
                            kind="ExternalInput").ap()
    vpiT_d = nc.dram_tensor("vpiT", [P, KCH * D], DT.bfloat16,
                            kind="ExternalInput").ap()
    ident_d = nc.dram_tensor("ident", [P, P], DT.bfloat16, kind="ExternalInput").ap()
    outT_d = nc.dram_tensor("outT", [P, NSH], DT.float32, kind="ExternalOutput").ap()
    zs_d = nc.dram_tensor("zs", [P, 2 * NC2], DT.float32, kind="ExternalOutput").ap()

    with ExitStack() as ctx:
        tc = ctx.enter_context(tile.TileContext(nc))
        cpool = ctx.enter_context(tc.tile_pool(name="const", bufs=1))
        smpool = ctx.enter_context(tc.tile_pool(name="sm", bufs=1))
        wpool = ctx.enter_context(tc.tile_pool(name="work", bufs=3))
        zpool = ctx.enter_context(tc.tile_pool(name="z", bufs=1))

        t2 = cpool.tile([NROW, NSH], DT.float16)
        si = cpool.tile([NROW, KPAD], DT.float16)
        qh = cpool.tile([P, NSH], DT.float16)
        ql = cpool.tile([P, NSH], DT.float16)
        ah = cpool.tile([P, AWK], DT.float16)
        al = cpool.tile([P, AWK], DT.float16)
        vprT = cpool.tile([P, KCH * D], DT.bfloat16)
        vpiT = cpool.tile([P, KCH * D], DT.bfloat16)
        ident = cpool.tile([P, P], DT.bfloat16)
        nc.sync.dma_start(t2[:], t2_d)
        nc.sync.dma_start(si[:], si_d)
        nc.sync.dma_start(qh[:], qh_d)
        nc.sync.dma_start(ql[:], ql_d)
        nc.sync.dma_start(ah[:], ah_d)
        nc.sync.dma_start(al[:], al_d)
        nc.sync.dma_start(vprT[:], vprT_d)
        nc.sync.dma_start(vpiT[:], vpiT_d)
        nc.sync.dma_start(ident[:], ident_d)

        sm = smpool.tile([P, NC2 * 4096], DT.bfloat16)
        zacc = zpool.tile([P, 2 * NC2], DT.float32)

        # ---- pass 1: aw (3-term fp16) -> softmax (2 halves per n-chunk)
        with tc.tile_pool(name="awps", bufs=1, space="PSUM") as awps:
            for c8 in range(NC2):
                nsl = slice(c8 * P, (c8 + 1) * P)
                awh = []
                mxs = []
                for h in range(2):
                    aw = awps.tile([P, 2048], DT.float32, tag=f"aw{h}")
                    for j in range(4):
                        ksl = slice(h * 2048 + j * 512, h * 2048 + (j + 1) * 512)
                        osl = slice(j * 512, (j + 1) * 512)
                        nc.tensor.matmul(aw[:, osl], qh[:, nsl], ah[:, ksl],
                                         start=True, stop=False)
                        nc.tensor.matmul(aw[:, osl], qh[:, nsl], al[:, ksl],
                                         start=False, stop=False)
                        nc.tensor.matmul(aw[:, osl], ql[:, nsl], ah[:, ksl],
                                         start=False, stop=True)
                    mx = wpool.tile([P, 1], DT.float32, tag=f"mx{h}")
                    nc.vector.reduce_max(mx[:], aw[:], axis=AX.X)
                    awh.append(aw)
                    mxs.append(mx)
                mxc = wpool.tile([P, 1], DT.float32, tag="mxc")
                nc.vector.tensor_tensor(mxc[:], mxs[0][:], mxs[1][:], ALU.max)
                negmx = wpool.tile([P, 1], DT.float32, tag="negmx")
                nc.vector.tensor_scalar_mul(negmx[:], mxc[:], -1.0)
                for h in range(2):
                    nc.scalar.activation(
                        sm[:, c8 * 4096 + h * 2048: c8 * 4096 + (h + 1) * 2048],
                        awh[h][:], F.Exp, bias=negmx[:],
                        accum_out=zacc[:, 2 * c8 + h: 2 * c8 + h + 1])

        # ---- pass 2: phases -> sin/cos -> PE-transpose sm -> inverse GEMM
        with (tc.tile_pool(name="php", bufs=1, space="PSUM") as php,
              tc.tile_pool(name="smtp", bufs=2, space="PSUM") as smtp,
              tc.tile_pool(name="ops", bufs=1, space="PSUM") as ops):
            outT = ops.tile([P, NSH], DT.float32)
            for kc in range(KCH):
                ph = php.tile([P, 1024], DT.float32, tag="ph")
                for h in range(2):
                    sl = slice(h * 512, (h + 1) * 512)
                    nc.tensor.matmul(ph[:, sl], si[:, kc * P:(kc + 1) * P],
                                     t2[:, sl], start=True, stop=True)
                r = wpool.tile([P, 1024], DT.float32, tag="r")
                nc.vector.add_range_wrap(r[:], ph[:], 0.0, 0.5, 1.0)
                w2 = wpool.tile([P, 1024], DT.float32, tag="w2")
                nc.vector.add_range_wrap(w2[:], r[:], 0.25, 0.5, 1.0)
                sint = wpool.tile([P, 1024], DT.bfloat16, tag="sint")
                cost = wpool.tile([P, 1024], DT.bfloat16, tag="cost")
                nc.scalar.activation(sint[:], r[:], F.Sin, scale=TWOPI)
                nc.scalar.activation(cost[:], w2[:], F.Sin, scale=TWOPI)
                smT = smtp.tile([P, 1024], DT.bfloat16, tag="smT")
                for c8 in range(NC2):
                    nc.tensor.transpose(
                        smT[:, c8 * P:(c8 + 1) * P],
                        sm[:, c8 * 4096 + kc * P: c8 * 4096 + (kc + 1) * P],
                        ident[:])
                smC = wpool.tile([P, 1024], DT.bfloat16, tag="smC")
                smS = wpool.tile([P, 1024], DT.bfloat16, tag="smS")
                nc.vector.tensor_mul(smC[:], smT[:], cost[:])
                nc.vector.tensor_mul(smS[:], smT[:], sint[:])
                dsl = slice(kc * D, (kc + 1) * D)
                for h in range(2):
                    sl = slice(h * 512, (h + 1) * 512)
                    nc.tensor.matmul(outT[:, sl], vprT[:, dsl], smC[:, sl],
                                     start=(kc == 0), stop=False)
                    nc.tensor.matmul(outT[:, sl], vpiT[:, dsl], smS[:, sl],
                                     start=False, stop=(kc == KCH - 1))

            res = wpool.tile([P, NSH], DT.float32, tag="res")
            nc.vector.tensor_copy(res[:], outT[:])
            nc.sync.dma_start(outT_d, res[:])
            nc.sync.dma_start(zs_d, zacc[:])

    nc.compile()
    return nc


# ---------------------------------------------------------------- profiling
def enable_ntff_profiling():
    """Provide the antenv.axon_hooks module run_bass_kernel_spmd needs for
    trace=True under axon, backed by trn_boot's ctypes NTFF hook."""
    import types
    if "antenv.axon_hooks" in sys.modules:
        return True
    sys.path.insert(0, "/root/.axon_site")
    try:
        from trn_agent_boot.trn_boot import _ntff_profile_via_ctypes
        hook = _ntff_profile_via_ctypes("/opt/axon/libaxon_pjrt.so")
    except Exception as e:
        print(f"ntff hook unavailable: {e}")
        return False
    if hook is None:
        print("ntff hook: .so lacks axon_start_nrt_profile")
        return False
    mod = types.ModuleType("antenv.axon_hooks")
    mod._hook = hook
    mod.get_axon_ntff_profile_hook = lambda: mod._hook
    mod.set_axon_ntff_profile_hook = lambda h: setattr(mod, "_hook", h)
    sys.modules["antenv.axon_hooks"] = mod
    # upload_artifacts copies the NEFF dir to a remote bucket -- hangs in
    # this container; keep artifacts local instead.
    import concourse.bass_utils as bu
    bu.upload_artifacts = lambda tmpdir: tmpdir
    return True


# ---------------------------------------------------------------- runner
_NC1 = None
_NC2 = None


def run_ewald(q_vector, k_vector, v_vector, positions, cell, batch, k_fwd,
              k_inv, trace=False):
    global _NC1, _NC2
    if trace:
        trace = enable_ntff_profiling()
    th, tl, sf, si, kvh, kvl, vvh, qh, ql = host_prep(
        q_vector, k_vector, v_vector, positions, cell, k_fwd, k_inv)

    kvh_c = chunk_major(kvh)
    kvl_c = chunk_major(kvl)
    vvh_c = chunk_major(vvh)

    if _NC1 is None:
        _NC1 = build_k1()
    in1 = [{"th": th, "tl": tl,
            "sf": np.ascontiguousarray(sf[:, c * KSH:(c + 1) * KSH]),
            "kvh": kvh_c, "kvl": kvl_c, "vvh": vvh_c} for c in range(8)]
    r1 = run_bass_kernel_spmd(_NC1, in1, list(range(8)), trace=trace)

    K = k_fwd.shape[0]
    kre = np.concatenate([r1.results[c]["kre"] for c in range(8)], axis=1)
    kim = np.concatenate([r1.results[c]["kim"] for c in range(8)], axis=1)
    vre = np.concatenate(
        [r1.results[c]["vre"].astype(np.float32) for c in range(8)], axis=1)
    vim = np.concatenate(
        [r1.results[c]["vim"].astype(np.float32) for c in range(8)], axis=1)
    akp = np.zeros((D, AWK), dtype=np.float32)
    akp[:, :KPAD] = np.hypot(kre, kim)
    akp[:, K:] = 0.0
    ah, al = split16(akp)
    vprT = chunk_major(np.ascontiguousarray(vre.T).astype(bf16))  # [P, KCH*D]
    vpiT = chunk_major(np.ascontiguousarray(vim.T).astype(bf16))
    ident = np.eye(P, dtype=np.float32).astype(bf16)

    if _NC2 is None:
        _NC2 = build_k2()
    in2 = [{"t2": np.ascontiguousarray(th[:, c * NSH:(c + 1) * NSH]),
            "si": si,
            "qh": np.ascontiguousarray(qh[:, c * NSH:(c + 1) * NSH]),
            "ql": np.ascontiguousarray(ql[:, c * NSH:(c + 1) * NSH]),
            "ah": ah, "al": al, "vprT": vprT, "vpiT": vpiT, "ident": ident}
           for c in range(8)]
    r2 = run_bass_kernel_spmd(_NC2, in2, list(range(8)), trace=trace)

    outs = []
    for c in range(8):
        oT = r2.results[c]["outT"]                    # [128 d, 1024 n]
        zs = r2.results[c]["zs"]                      # [128, 16]
        z = (zs[:, 0::2] + zs[:, 1::2]).T.reshape(-1)  # atom n=c8*128+p
        outs.append((oT.T / z[:, None]).astype(np.float32))
    out = np.concatenate(outs, axis=0)
    return out, (r1, r2)


# ---------------------------------------------------------------- entry point
def kernel(q_vector, k_vector, v_vector, positions, cell, batch, k_fwd, k_inv):
    """Full-input entry: shards across 8 NeuronCores internally."""
    out, _ = run_ewald(np.asarray(q_vector), np.asarray(k_vector),
                       np.asarray(v_vector), np.asarray(positions),
                       np.asarray(cell), np.asarray(batch),
                       np.asarray(k_fwd), np.asarray(k_inv))
    return out


# revision 8
# speedup vs baseline: 1.8868x; 1.1210x over previous
"""Ewald potential Bass kernels for TRN2 (8-core SPMD), v2.

K1 shards k-space (480 cols/core of padded 3840) over all 8192 atoms ->
k_pot re/im (fp32) and v_pot re/im (fp16). Host gathers, computes
akp=|k_pot| and fp16 splits. K2 shards atoms (1024/core): aw GEMM (3-term
fp16 split) -> softmax -> inverse transform via PE-transposed sm.

Phases come from a one-hot selection GEMM against host-precomputed
frac(coord*k) tables centered in [-0.5,0.5]: phase' = Tx+Ty+Tz in
[-1.5,1.5], range-reduced with a single ADD_RANGE_WRAP, cos via a second
wrap (+0.25). Sin activation with scale=2pi.

out[n,d] = sum_k sm[n,k] * (cos(2pi phi_i)*vpr[k,d] + sin(2pi phi_i)*vpi[k,d]) / Z[n]
with eik_i = exp(-2pi i phi_i) = cos - i sin.
"""
import sys
sys.path.insert(0, '/opt/trn_rl_repo')
import numpy as np
import ml_dtypes
import concourse.bass as bass
import concourse.tile as tile
import concourse.mybir as mybir
from concourse import bacc
from concourse.bass_utils import run_bass_kernel_spmd
from contextlib import ExitStack

F = mybir.ActivationFunctionType
DT = mybir.dt
ALU = mybir.AluOpType
AX = mybir.AxisListType

P = 128
N = 8192
D = 128
NK = 12              # grid: kx in [0,12], ky/kz in [-12,12]
KPAD = 3840          # 3796 padded to 30*128
KSH = KPAD // 8      # 480 k-cols per core in K1
NSH = N // 8         # 1024 atoms per core in K2
NCH = N // P         # 64 atom chunks in K1
KCH = KPAD // P      # 30 k chunks in K2
AWK = 4096           # aw/sm width per n-chunk (2 halves of 2048)
NC2 = NSH // P       # 8 atom chunks in K2
NROW = 63            # 13 x-rows + 25 y-rows + 25 z-rows
TWOPI = float(2 * np.pi)

bf16 = ml_dtypes.bfloat16
f16 = np.float16


def _frac_tables(rfrac):
    """[63, n] fp64 tables: frac(coord*u) centered to [-0.5, 0.5]."""
    n = rfrac.shape[0]
    t = np.zeros((NROW, n), dtype=np.float64)
    r64 = rfrac.astype(np.float64)
    for u in range(NK + 1):                      # x rows: u = 0..12
        v = r64[:, 0] * u
        t[u] = v - np.round(v)
    for i, u in enumerate(range(-NK, NK + 1)):   # y rows
        v = r64[:, 1] * u
        t[13 + i] = v - np.round(v)
    for i, u in enumerate(range(-NK, NK + 1)):   # z rows
        v = r64[:, 2] * u
        t[38 + i] = v - np.round(v)
    return t


def _select_mat(kmat):
    """[63, KPAD] fp16 one-hot selection for k rows (padded cols zero)."""
    K = kmat.shape[0]
    s = np.zeros((NROW, KPAD), dtype=np.float32)
    j = np.arange(K)
    s[kmat[:, 0], j] = 1.0
    s[13 + kmat[:, 1] + NK, j] = 1.0
    s[38 + kmat[:, 2] + NK, j] = 1.0
    return s.astype(f16)


def split16(x):
    """fp16 2-way split: x ~ hi + lo to ~2^-22 rel."""
    x = np.asarray(x, dtype=np.float32)
    hi = x.astype(f16)
    lo = (x - hi.astype(np.float32)).astype(f16)
    return hi, lo


def host_prep(q_vector, k_vector, v_vector, positions, cell, k_fwd, k_inv):
    L = float(np.asarray(cell).reshape(3, 3)[0, 0])
    rfrac = np.asarray(positions, dtype=np.float64) / L
    t64 = _frac_tables(rfrac)                     # [63, N]
    th = t64.astype(f16)
    tl = (t64 - th.astype(np.float64)).astype(f16)
    sf = _select_mat(np.asarray(k_fwd))           # [63, KPAD]
    si = _select_mat(np.asarray(k_inv))
    kvh, kvl = split16(k_vector)                  # [N, D]
    vvh = np.asarray(v_vector, dtype=np.float32).astype(f16)
    qh, ql = split16(np.abs(np.asarray(q_vector, dtype=np.float32)).T)  # [D, N]
    return th, tl, sf, si, kvh, kvl, vvh, qh, ql


def chunk_major(x):
    """[N, D] -> [P, NCH*D]: partition=n%P? No: chunk c rows c*P..(c+1)*P
    land at [:, c*D:(c+1)*D]."""
    n, d = x.shape
    c = n // P
    return np.ascontiguousarray(
        x.reshape(c, P, d).transpose(1, 0, 2).reshape(P, c * d))


# ---------------------------------------------------------------- kernel 1
def build_k1():
    nc = bacc.Bacc("TRN2", target_bir_lowering=False, debug=False)
    th_d = nc.dram_tensor("th", [NROW, N], DT.float16, kind="ExternalInput").ap()
    tl_d = nc.dram_tensor("tl", [NROW, N], DT.float16, kind="ExternalInput").ap()
    sf_d = nc.dram_tensor("sf", [NROW, KSH], DT.float16, kind="ExternalInput").ap()
    kvh_d = nc.dram_tensor("kvh", [P, NCH * D], DT.float16, kind="ExternalInput").ap()
    vvh_d = nc.dram_tensor("vvh", [P, NCH * D], DT.float16, kind="ExternalInput").ap()
    kre_d = nc.dram_tensor("kre", [P, KSH], DT.float32, kind="ExternalOutput").ap()
    kim_d = nc.dram_tensor("kim", [P, KSH], DT.float32, kind="ExternalOutput").ap()
    vre_d = nc.dram_tensor("vre", [P, KSH], DT.float16, kind="ExternalOutput").ap()
    vim_d = nc.dram_tensor("vim", [P, KSH], DT.float16, kind="ExternalOutput").ap()

    with ExitStack() as ctx:
        tc = ctx.enter_context(tile.TileContext(nc))
        cpool = ctx.enter_context(tc.tile_pool(name="const", bufs=1))
        wpool = ctx.enter_context(tc.tile_pool(name="work", bufs=3))
        php = ctx.enter_context(tc.tile_pool(name="ph", bufs=4, space="PSUM"))
        accp = ctx.enter_context(tc.tile_pool(name="acc", bufs=1, space="PSUM"))

        th = cpool.tile([NROW, N], DT.float16)
        tlo = cpool.tile([NROW, N], DT.float16)
        sf = cpool.tile([NROW, KSH], DT.float16)
        kvh = cpool.tile([P, NCH * D], DT.float16)
        vvh = cpool.tile([P, NCH * D], DT.float16)
        nc.sync.dma_start(sf[:], sf_d)
        nc.sync.dma_start(th[:], th_d)
        nc.sync.dma_start(tlo[:], tl_d)
        nc.sync.dma_start(kvh[:], kvh_d)
        nc.sync.dma_start(vvh[:], vvh_d)

        kre = accp.tile([P, 512], DT.float32)
        kim = accp.tile([P, 512], DT.float32)
        vre = accp.tile([P, 512], DT.float32)
        vim = accp.tile([P, 512], DT.float32)

        phs = {}

        def emit_ph(c):
            if c >= NCH:
                return
            t = php.tile([P, 512], DT.float32, tag="ph")
            nc.tensor.matmul(t[:, :KSH], th[:, c * P:(c + 1) * P], sf[:],
                             start=True, stop=False)
            nc.tensor.matmul(t[:, :KSH], tlo[:, c * P:(c + 1) * P], sf[:],
                             start=False, stop=True)
            phs[c] = t

        def emit_acc(p, sinf, cosf):
            # cos-consumers first: kre/vre, then kim/vim
            for h in range(2):
                c = 2 * p + h
                sl = slice(h * 512, h * 512 + KSH)
                dsl = slice(c * D, (c + 1) * D)
                nc.tensor.matmul(kre[:, :KSH], kvh[:, dsl], cosf[:, sl],
                                 start=(c == 0), stop=(c == NCH - 1))
                nc.tensor.matmul(vre[:, :KSH], vvh[:, dsl], cosf[:, sl],
                                 start=(c == 0), stop=(c == NCH - 1))
            for h in range(2):
                c = 2 * p + h
                sl = slice(h * 512, h * 512 + KSH)
                dsl = slice(c * D, (c + 1) * D)
                nc.tensor.matmul(kim[:, :KSH], kvh[:, dsl], sinf[:, sl],
                                 start=(c == 0), stop=(c == NCH - 1))
                nc.tensor.matmul(vim[:, :KSH], vvh[:, dsl], sinf[:, sl],
                                 start=(c == 0), stop=(c == NCH - 1))

        for c in range(4):
            emit_ph(c)
        prev = None          # (sinf, cosf) of pair p-1
        for p in range(NCH // 2):
            a, b = 2 * p, 2 * p + 1
            r = wpool.tile([P, 1024], DT.float32, tag="r")
            w2 = wpool.tile([P, 1024], DT.float32, tag="w2")
            nc.vector.add_range_wrap(r[:, 0:512], phs[a][:], 0.0, 0.5, 1.0)
            nc.vector.add_range_wrap(w2[:, 0:512], r[:, 0:512], 0.25, 0.5, 1.0)
            nc.vector.add_range_wrap(r[:, 512:1024], phs[b][:], 0.0, 0.5, 1.0)
            nc.vector.add_range_wrap(w2[:, 512:1024], r[:, 512:1024], 0.25, 0.5,
                                     1.0)
            del phs[a], phs[b]
            emit_ph(2 * p + 4)
            emit_ph(2 * p + 5)
            sinf = wpool.tile([P, 1024], DT.float16, tag="sinf")
            cosf = wpool.tile([P, 1024], DT.float16, tag="cosf")
            nc.scalar.activation(cosf[:], w2[:], F.Sin, scale=TWOPI)
            nc.scalar.activation(sinf[:], r[:], F.Sin, scale=TWOPI)
            if prev is not None:
                emit_acc(p - 1, *prev)
            prev = (sinf, cosf)
        emit_acc(NCH // 2 - 1, *prev)

        kre_s = wpool.tile([P, KSH], DT.float32, tag="kre_s")
        kim_s = wpool.tile([P, KSH], DT.float32, tag="kim_s")
        vre_s = wpool.tile([P, KSH], DT.float16, tag="vre_s")
        vim_s = wpool.tile([P, KSH], DT.float16, tag="vim_s")
        nc.vector.tensor_copy(kre_s[:], kre[:, :KSH])
        nc.vector.tensor_copy(kim_s[:], kim[:, :KSH])
        nc.vector.tensor_copy(vre_s[:], vre[:, :KSH])
        nc.vector.tensor_copy(vim_s[:], vim[:, :KSH])
        nc.sync.dma_start(kre_d, kre_s[:])
        nc.sync.dma_start(kim_d, kim_s[:])
        nc.sync.dma_start(vre_d, vre_s[:])
        nc.sync.dma_start(vim_d, vim_s[:])

    nc.compile()
    return nc


# ---------------------------------------------------------------- kernel 2
def build_k2():
    nc = bacc.Bacc("TRN2", target_bir_lowering=False, debug=False)
    t2_d = nc.dram_tensor("t2", [NROW, NSH], DT.float16, kind="ExternalInput").ap()
    si_d = nc.dram_tensor("si", [NROW, KPAD], DT.float16, kind="ExternalInput").ap()
    qh_d = nc.dram_tensor("qh", [P, NSH], DT.float16, kind="ExternalInput").ap()
    ah_d = nc.dram_tensor("ah", [P, AWK], DT.float16, kind="ExternalInput").ap()
    al_d = nc.dram_tensor("al", [P, AWK], DT.float16, kind="ExternalInput").ap()
    vprT_d = nc.dram_tensor("vprT", [P, KCH * D], DT.bfloat16,
                            kind="ExternalInput").ap()
    vpiT_d = nc.dram_tensor("vpiT", [P, KCH * D], DT.bfloat16,
                            kind="ExternalInput").ap()
    ident_d = nc.dram_tensor("ident", [P, P], DT.bfloat16, kind="ExternalInput").ap()
    outT_d = nc.dram_tensor("outT", [P, NSH], DT.float32, kind="ExternalOutput").ap()
    zs_d = nc.dram_tensor("zs", [P, 2 * NC2], DT.float32, kind="ExternalOutput").ap()

    with ExitStack() as ctx:
        tc = ctx.enter_context(tile.TileContext(nc))
        cpool = ctx.enter_context(tc.tile_pool(name="const", bufs=1))
        smpool = ctx.enter_context(tc.tile_pool(name="sm", bufs=1))
        wpool = ctx.enter_context(tc.tile_pool(name="work", bufs=3))
        zpool = ctx.enter_context(tc.tile_pool(name="z", bufs=1))

        t2 = cpool.tile([NROW, NSH], DT.float16)
        si = cpool.tile([NROW, KPAD], DT.float16)
        qh = cpool.tile([P, NSH], DT.float16)
        ah = cpool.tile([P, AWK], DT.float16)
        al = cpool.tile([P, AWK], DT.float16)
        vprT = cpool.tile([P, KCH * D], DT.bfloat16)
        vpiT = cpool.tile([P, KCH * D], DT.bfloat16)
        ident = cpool.tile([P, P], DT.bfloat16)
        nc.sync.dma_start(t2[:], t2_d)
        nc.sync.dma_start(si[:], si_d)
        nc.sync.dma_start(qh[:], qh_d)
        nc.sync.dma_start(ah[:], ah_d)
        nc.sync.dma_start(al[:], al_d)
        nc.sync.dma_start(vprT[:], vprT_d)
        nc.sync.dma_start(vpiT[:], vpiT_d)
        nc.sync.dma_start(ident[:], ident_d)

        sm = smpool.tile([P, NC2 * 4096], DT.bfloat16)
        zacc = zpool.tile([P, 2 * NC2], DT.float32)

        # ---- pass 1: aw (3-term fp16) -> softmax (2 halves per n-chunk)
        with tc.tile_pool(name="awps", bufs=1, space="PSUM") as awps:
            for c8 in range(NC2):
                nsl = slice(c8 * P, (c8 + 1) * P)
                awh = []
                mxs = []
                for h in range(2):
                    aw = awps.tile([P, 2048], DT.float32, tag=f"aw{h}")
                    for j in range(4):
                        ksl = slice(h * 2048 + j * 512, h * 2048 + (j + 1) * 512)
                        osl = slice(j * 512, (j + 1) * 512)
                        nc.tensor.matmul(aw[:, osl], qh[:, nsl], ah[:, ksl],
                                         start=True, stop=False)
                        nc.tensor.matmul(aw[:, osl], qh[:, nsl], al[:, ksl],
                                         start=False, stop=True)
                    mx = wpool.tile([P, 1], DT.float32, tag=f"mx{h}")
                    nc.vector.reduce_max(mx[:], aw[:], axis=AX.X)
                    awh.append(aw)
                    mxs.append(mx)
                mxc = wpool.tile([P, 1], DT.float32, tag="mxc")
                nc.vector.tensor_tensor(mxc[:], mxs[0][:], mxs[1][:], ALU.max)
                negmx = wpool.tile([P, 1], DT.float32, tag="negmx")
                nc.vector.tensor_scalar_mul(negmx[:], mxc[:], -1.0)
                for h in range(2):
                    nc.scalar.activation(
                        sm[:, c8 * 4096 + h * 2048: c8 * 4096 + (h + 1) * 2048],
                        awh[h][:], F.Exp, bias=negmx[:],
                        accum_out=zacc[:, 2 * c8 + h: 2 * c8 + h + 1])

        # ---- pass 2: phases -> sin/cos -> PE-transpose sm -> inverse GEMM
        with (tc.tile_pool(name="php", bufs=4, space="PSUM") as php,
              tc.tile_pool(name="smtp", bufs=2, space="PSUM") as smtp,
              tc.tile_pool(name="ops", bufs=1, space="PSUM") as ops):
            outT = ops.tile([P, NSH], DT.float32)
            phs = {}

            def emit_ph(j):          # j = half index 0..59 (2 per k-chunk)
                if j >= 2 * KCH:
                    return
                t = php.tile([P, 512], DT.float32, tag="ph")
                nc.tensor.matmul(t[:], si[:, (j // 2) * P:(j // 2 + 1) * P],
                                 t2[:, (j % 2) * 512:(j % 2 + 1) * 512],
                                 start=True, stop=True)
                phs[j] = t

            def emit_inv(kc, smC, smS):
                dsl = slice(kc * D, (kc + 1) * D)
                for h in range(2):
                    sl = slice(h * 512, (h + 1) * 512)
                    nc.tensor.matmul(outT[:, sl], vprT[:, dsl], smC[:, sl],
                                     start=(kc == 0), stop=False)
                    nc.tensor.matmul(outT[:, sl], vpiT[:, dsl], smS[:, sl],
                                     start=False, stop=(kc == KCH - 1))

            for j in range(4):
                emit_ph(j)
            prev = None
            for kc in range(KCH):
                r = wpool.tile([P, 1024], DT.float32, tag="r")
                w2 = wpool.tile([P, 1024], DT.float32, tag="w2")
                for h in range(2):
                    j = 2 * kc + h
                    sl = slice(h * 512, (h + 1) * 512)
                    nc.vector.add_range_wrap(r[:, sl], phs[j][:], 0.0, 0.5, 1.0)
                    nc.vector.add_range_wrap(w2[:, sl], r[:, sl], 0.25, 0.5, 1.0)
                    del phs[j]
                emit_ph(2 * kc + 4)
                emit_ph(2 * kc + 5)
                smT = smtp.tile([P, 1024], DT.bfloat16, tag="smT")
                for c8 in range(NC2):
                    nc.tensor.transpose(
                        smT[:, c8 * P:(c8 + 1) * P],
                        sm[:, c8 * AWK + kc * P: c8 * AWK + (kc + 1) * P],
                        ident[:])
                sint = wpool.tile([P, 1024], DT.bfloat16, tag="sint")
                cost = wpool.tile([P, 1024], DT.bfloat16, tag="cost")
                nc.scalar.activation(cost[:], w2[:], F.Sin, scale=TWOPI)
                nc.scalar.activation(sint[:], r[:], F.Sin, scale=TWOPI)
                smC = wpool.tile([P, 1024], DT.bfloat16, tag="smC")
                smS = wpool.tile([P, 1024], DT.bfloat16, tag="smS")
                nc.vector.tensor_mul(smC[:], smT[:], cost[:])
                nc.vector.tensor_mul(smS[:], smT[:], sint[:])
                if prev is not None:
                    emit_inv(kc - 1, *prev)
                prev = (smC, smS)
            emit_inv(KCH - 1, *prev)

            res = wpool.tile([P, NSH], DT.float32, tag="res")
            nc.vector.tensor_copy(res[:], outT[:])
            nc.sync.dma_start(outT_d, res[:])
            nc.sync.dma_start(zs_d, zacc[:])

    nc.compile()
    return nc


# ---------------------------------------------------------------- profiling
def enable_ntff_profiling():
    """Provide the antenv.axon_hooks module run_bass_kernel_spmd needs for
    trace=True under axon, backed by trn_boot's ctypes NTFF hook."""
    import types
    if "antenv.axon_hooks" in sys.modules:
        return True
    sys.path.insert(0, "/root/.axon_site")
    try:
        from trn_agent_boot.trn_boot import _ntff_profile_via_ctypes
        hook = _ntff_profile_via_ctypes("/opt/axon/libaxon_pjrt.so")
    except Exception as e:
        print(f"ntff hook unavailable: {e}")
        return False
    if hook is None:
        print("ntff hook: .so lacks axon_start_nrt_profile")
        return False
    mod = types.ModuleType("antenv.axon_hooks")
    mod._hook = hook
    mod.get_axon_ntff_profile_hook = lambda: mod._hook
    mod.set_axon_ntff_profile_hook = lambda h: setattr(mod, "_hook", h)
    sys.modules["antenv.axon_hooks"] = mod
    # upload_artifacts copies the NEFF dir to a remote bucket -- hangs in
    # this container; keep artifacts local instead.
    import concourse.bass_utils as bu
    bu.upload_artifacts = lambda tmpdir: tmpdir
    return True


# ---------------------------------------------------------------- runner
_NC1 = None
_NC2 = None


def run_ewald(q_vector, k_vector, v_vector, positions, cell, batch, k_fwd,
              k_inv, trace=False):
    global _NC1, _NC2
    if trace:
        trace = enable_ntff_profiling()
    th, tl, sf, si, kvh, kvl, vvh, qh, ql = host_prep(
        q_vector, k_vector, v_vector, positions, cell, k_fwd, k_inv)

    kvh_c = chunk_major(kvh)
    vvh_c = chunk_major(vvh)

    if _NC1 is None:
        _NC1 = build_k1()
    in1 = [{"th": th, "tl": tl,
            "sf": np.ascontiguousarray(sf[:, c * KSH:(c + 1) * KSH]),
            "kvh": kvh_c, "vvh": vvh_c} for c in range(8)]
    r1 = run_bass_kernel_spmd(_NC1, in1, list(range(8)), trace=trace)

    K = k_fwd.shape[0]
    kre = np.concatenate([r1.results[c]["kre"] for c in range(8)], axis=1)
    kim = np.concatenate([r1.results[c]["kim"] for c in range(8)], axis=1)
    vre = np.concatenate(
        [r1.results[c]["vre"].astype(np.float32) for c in range(8)], axis=1)
    vim = np.concatenate(
        [r1.results[c]["vim"].astype(np.float32) for c in range(8)], axis=1)
    akp = np.zeros((D, AWK), dtype=np.float32)
    akp[:, :KPAD] = np.hypot(kre, kim)
    akp[:, K:] = 0.0
    ah, al = split16(akp)
    vprT = chunk_major(np.ascontiguousarray(vre.T).astype(bf16))  # [P, KCH*D]
    vpiT = chunk_major(np.ascontiguousarray(vim.T).astype(bf16))
    ident = np.eye(P, dtype=np.float32).astype(bf16)

    if _NC2 is None:
        _NC2 = build_k2()
    in2 = [{"t2": np.ascontiguousarray(th[:, c * NSH:(c + 1) * NSH]),
            "si": si,
            "qh": np.ascontiguousarray(qh[:, c * NSH:(c + 1) * NSH]),
            "ah": ah, "al": al, "vprT": vprT, "vpiT": vpiT, "ident": ident}
           for c in range(8)]
    r2 = run_bass_kernel_spmd(_NC2, in2, list(range(8)), trace=trace)

    outs = []
    for c in range(8):
        oT = r2.results[c]["outT"]                    # [128 d, 1024 n]
        zs = r2.results[c]["zs"]                      # [128, 16]
        z = (zs[:, 0::2] + zs[:, 1::2]).T.reshape(-1)  # atom n=c8*128+p
        outs.append((oT.T / z[:, None]).astype(np.float32))
    out = np.concatenate(outs, axis=0)
    return out, (r1, r2)


# ---------------------------------------------------------------- entry point
def kernel(q_vector, k_vector, v_vector, positions, cell, batch, k_fwd, k_inv):
    """Full-input entry: shards across 8 NeuronCores internally."""
    out, _ = run_ewald(np.asarray(q_vector), np.asarray(k_vector),
                       np.asarray(v_vector), np.asarray(positions),
                       np.asarray(cell), np.asarray(batch),
                       np.asarray(k_fwd), np.asarray(k_inv))
    return out


# revision 11
# speedup vs baseline: 1.9800x; 1.0494x over previous
"""Ewald potential Bass kernels for TRN2 (8-core SPMD), v2.

K1 shards k-space (480 cols/core of padded 3840) over all 8192 atoms ->
k_pot re/im (fp32) and v_pot re/im (fp16). Host gathers, computes
akp=|k_pot| and fp16 splits. K2 shards atoms (1024/core): aw GEMM (3-term
fp16 split) -> softmax -> inverse transform via PE-transposed sm.

Phases come from a one-hot selection GEMM against host-precomputed
frac(coord*k) tables centered in [-0.5,0.5]: phase' = Tx+Ty+Tz in
[-1.5,1.5], range-reduced with a single ADD_RANGE_WRAP, cos via a second
wrap (+0.25). Sin activation with scale=2pi.

out[n,d] = sum_k sm[n,k] * (cos(2pi phi_i)*vpr[k,d] + sin(2pi phi_i)*vpi[k,d]) / Z[n]
with eik_i = exp(-2pi i phi_i) = cos - i sin.
"""
import sys
sys.path.insert(0, '/opt/trn_rl_repo')
import numpy as np
import ml_dtypes
import concourse.bass as bass
import concourse.tile as tile
import concourse.mybir as mybir
from concourse import bacc
from concourse.bass_utils import run_bass_kernel_spmd
from contextlib import ExitStack

F = mybir.ActivationFunctionType
DT = mybir.dt
ALU = mybir.AluOpType
AX = mybir.AxisListType

P = 128
N = 8192
D = 128
NK = 12              # grid: kx in [0,12], ky/kz in [-12,12]
KPAD = 3840          # 3796 padded to 30*128
KSH = KPAD // 8      # 480 k-cols per core in K1
NSH = N // 8         # 1024 atoms per core in K2
NCH = N // P         # 64 atom chunks in K1
KCH = KPAD // P      # 30 k chunks in K2
AWK = 4096           # aw/sm width per n-chunk (2 halves of 2048)
NC2 = NSH // P       # 8 atom chunks in K2
NROW = 63            # 13 x-rows + 25 y-rows + 25 z-rows
TWOPI = float(2 * np.pi)

bf16 = ml_dtypes.bfloat16
f16 = np.float16


def _frac_tables(rfrac):
    """[63, n] fp64 tables: frac(coord*u) centered to [-0.5, 0.5]."""
    n = rfrac.shape[0]
    t = np.zeros((NROW, n), dtype=np.float64)
    r64 = rfrac.astype(np.float64)
    for u in range(NK + 1):                      # x rows: u = 0..12
        v = r64[:, 0] * u
        t[u] = v - np.round(v)
    for i, u in enumerate(range(-NK, NK + 1)):   # y rows
        v = r64[:, 1] * u
        t[13 + i] = v - np.round(v)
    for i, u in enumerate(range(-NK, NK + 1)):   # z rows
        v = r64[:, 2] * u
        t[38 + i] = v - np.round(v)
    return t


def _select_mat(kmat):
    """[63, KPAD] fp16 one-hot selection for k rows (padded cols zero)."""
    K = kmat.shape[0]
    s = np.zeros((NROW, KPAD), dtype=np.float32)
    j = np.arange(K)
    s[kmat[:, 0], j] = 1.0
    s[13 + kmat[:, 1] + NK, j] = 1.0
    s[38 + kmat[:, 2] + NK, j] = 1.0
    return s.astype(f16)


def split16(x):
    """fp16 2-way split: x ~ hi + lo to ~2^-22 rel."""
    x = np.asarray(x, dtype=np.float32)
    hi = x.astype(f16)
    lo = (x - hi.astype(np.float32)).astype(f16)
    return hi, lo


def host_prep(q_vector, k_vector, v_vector, positions, cell, k_fwd, k_inv):
    L = float(np.asarray(cell).reshape(3, 3)[0, 0])
    rfrac = np.asarray(positions, dtype=np.float64) / L
    t64 = _frac_tables(rfrac)                     # [63, N]
    th = t64.astype(f16)
    tl = (t64 - th.astype(np.float64)).astype(f16)
    sf = _select_mat(np.asarray(k_fwd))           # [63, KPAD]
    si = _select_mat(np.asarray(k_inv))
    kvh, kvl = split16(k_vector)                  # [N, D]
    vvh = np.asarray(v_vector, dtype=np.float32).astype(f16)
    qh, ql = split16(np.abs(np.asarray(q_vector, dtype=np.float32)).T)  # [D, N]
    return th, tl, sf, si, kvh, kvl, vvh, qh, ql


def chunk_major(x):
    """[N, D] -> [P, NCH*D]: partition=n%P? No: chunk c rows c*P..(c+1)*P
    land at [:, c*D:(c+1)*D]."""
    n, d = x.shape
    c = n // P
    return np.ascontiguousarray(
        x.reshape(c, P, d).transpose(1, 0, 2).reshape(P, c * d))


# ---------------------------------------------------------------- kernel 1
def build_k1():
    nc = bacc.Bacc("TRN2", target_bir_lowering=False, debug=False)
    th_d = nc.dram_tensor("th", [NROW, N], DT.float16, kind="ExternalInput").ap()
    tl_d = nc.dram_tensor("tl", [NROW, N], DT.float16, kind="ExternalInput").ap()
    sf_d = nc.dram_tensor("sf", [NROW, KSH], DT.float16, kind="ExternalInput").ap()
    kvh_d = nc.dram_tensor("kvh", [P, NCH * D], DT.float16, kind="ExternalInput").ap()
    vvh_d = nc.dram_tensor("vvh", [P, NCH * D], DT.float16, kind="ExternalInput").ap()
    kre_d = nc.dram_tensor("kre", [P, KSH], DT.float32, kind="ExternalOutput").ap()
    kim_d = nc.dram_tensor("kim", [P, KSH], DT.float32, kind="ExternalOutput").ap()
    vre_d = nc.dram_tensor("vre", [P, KSH], DT.float16, kind="ExternalOutput").ap()
    vim_d = nc.dram_tensor("vim", [P, KSH], DT.float16, kind="ExternalOutput").ap()

    with ExitStack() as ctx:
        tc = ctx.enter_context(tile.TileContext(nc))
        cpool = ctx.enter_context(tc.tile_pool(name="const", bufs=1))
        wpool = ctx.enter_context(tc.tile_pool(name="work", bufs=3))
        php = ctx.enter_context(tc.tile_pool(name="ph", bufs=4, space="PSUM"))
        accp = ctx.enter_context(tc.tile_pool(name="acc", bufs=1, space="PSUM"))

        th = cpool.tile([NROW, N], DT.float16)
        tlo = cpool.tile([NROW, N], DT.float16)
        sf = cpool.tile([NROW, KSH], DT.float16)
        kvh = cpool.tile([P, NCH * D], DT.float16)
        vvh = cpool.tile([P, NCH * D], DT.float16)
        nc.sync.dma_start(sf[:], sf_d)
        nc.sync.dma_start(th[:], th_d)
        nc.sync.dma_start(tlo[:], tl_d)
        nc.sync.dma_start(kvh[:], kvh_d)
        nc.sync.dma_start(vvh[:], vvh_d)

        kre = accp.tile([P, 512], DT.float32)
        kim = accp.tile([P, 512], DT.float32)
        vre = accp.tile([P, 512], DT.float32)
        vim = accp.tile([P, 512], DT.float32)

        phs = {}

        def emit_ph(c):
            if c >= NCH:
                return
            t = php.tile([P, 512], DT.float32, tag="ph")
            nc.tensor.matmul(t[:, :KSH], th[:, c * P:(c + 1) * P], sf[:],
                             start=True, stop=False)
            nc.tensor.matmul(t[:, :KSH], tlo[:, c * P:(c + 1) * P], sf[:],
                             start=False, stop=True)
            phs[c] = t

        def emit_acc(p, sinf, cosf):
            # cos-consumers first: kre/vre, then kim/vim
            for h in range(2):
                c = 2 * p + h
                sl = slice(h * 512, h * 512 + KSH)
                dsl = slice(c * D, (c + 1) * D)
                nc.tensor.matmul(kre[:, :KSH], kvh[:, dsl], cosf[:, sl],
                                 start=(c == 0), stop=(c == NCH - 1))
                nc.tensor.matmul(vre[:, :KSH], vvh[:, dsl], cosf[:, sl],
                                 start=(c == 0), stop=(c == NCH - 1))
            for h in range(2):
                c = 2 * p + h
                sl = slice(h * 512, h * 512 + KSH)
                dsl = slice(c * D, (c + 1) * D)
                nc.tensor.matmul(kim[:, :KSH], kvh[:, dsl], sinf[:, sl],
                                 start=(c == 0), stop=(c == NCH - 1))
                nc.tensor.matmul(vim[:, :KSH], vvh[:, dsl], sinf[:, sl],
                                 start=(c == 0), stop=(c == NCH - 1))

        for c in range(4):
            emit_ph(c)
        prev = None          # (sinf, cosf) of pair p-1
        for p in range(NCH // 2):
            a, b = 2 * p, 2 * p + 1
            r = wpool.tile([P, 1024], DT.float32, tag="r")
            w2 = wpool.tile([P, 1024], DT.float32, tag="w2")
            nc.vector.add_range_wrap(r[:, 0:512], phs[a][:], 0.0, 0.5, 1.0)
            nc.vector.add_range_wrap(w2[:, 0:512], r[:, 0:512], 0.25, 0.5, 1.0)
            nc.vector.add_range_wrap(r[:, 512:1024], phs[b][:], 0.0, 0.5, 1.0)
            nc.vector.add_range_wrap(w2[:, 512:1024], r[:, 512:1024], 0.25, 0.5,
                                     1.0)
            del phs[a], phs[b]
            emit_ph(2 * p + 4)
            emit_ph(2 * p + 5)
            sinf = wpool.tile([P, 1024], DT.float16, tag="sinf")
            cosf = wpool.tile([P, 1024], DT.float16, tag="cosf")
            nc.scalar.activation(cosf[:], w2[:], F.Sin, scale=TWOPI)
            nc.scalar.activation(sinf[:], r[:], F.Sin, scale=TWOPI)
            if prev is not None:
                emit_acc(p - 1, *prev)
            prev = (sinf, cosf)
        emit_acc(NCH // 2 - 1, *prev)

        kre_s = wpool.tile([P, KSH], DT.float32, tag="kre_s")
        kim_s = wpool.tile([P, KSH], DT.float32, tag="kim_s")
        vre_s = wpool.tile([P, KSH], DT.float16, tag="vre_s")
        vim_s = wpool.tile([P, KSH], DT.float16, tag="vim_s")
        nc.vector.tensor_copy(kre_s[:], kre[:, :KSH])
        nc.vector.tensor_copy(kim_s[:], kim[:, :KSH])
        nc.vector.tensor_copy(vre_s[:], vre[:, :KSH])
        nc.vector.tensor_copy(vim_s[:], vim[:, :KSH])
        nc.sync.dma_start(kre_d, kre_s[:])
        nc.sync.dma_start(kim_d, kim_s[:])
        nc.sync.dma_start(vre_d, vre_s[:])
        nc.sync.dma_start(vim_d, vim_s[:])

    nc.compile()
    return nc


# ---------------------------------------------------------------- kernel 2
def build_k2():
    nc = bacc.Bacc("TRN2", target_bir_lowering=False, debug=False)
    t2_d = nc.dram_tensor("t2", [NROW, NSH], DT.float16, kind="ExternalInput").ap()
    si_d = nc.dram_tensor("si", [NROW, KPAD], DT.float16, kind="ExternalInput").ap()
    qh_d = nc.dram_tensor("qh", [P, NSH], DT.float16, kind="ExternalInput").ap()
    ah_d = nc.dram_tensor("ah", [P, AWK], DT.float16, kind="ExternalInput").ap()
    al_d = nc.dram_tensor("al", [P, AWK], DT.float16, kind="ExternalInput").ap()
    vprT_d = nc.dram_tensor("vprT", [P, KCH * D], DT.bfloat16,
                            kind="ExternalInput").ap()
    vpiT_d = nc.dram_tensor("vpiT", [P, KCH * D], DT.bfloat16,
                            kind="ExternalInput").ap()
    ident_d = nc.dram_tensor("ident", [P, P], DT.bfloat16, kind="ExternalInput").ap()
    outT_d = nc.dram_tensor("outT", [P, NSH], DT.float32, kind="ExternalOutput").ap()
    zs_d = nc.dram_tensor("zs", [P, 4 * NC2], DT.float32, kind="ExternalOutput").ap()

    with ExitStack() as ctx:
        tc = ctx.enter_context(tile.TileContext(nc))
        cpool = ctx.enter_context(tc.tile_pool(name="const", bufs=1))
        smpool = ctx.enter_context(tc.tile_pool(name="sm", bufs=1))
        wpool = ctx.enter_context(tc.tile_pool(name="work", bufs=3))
        zpool = ctx.enter_context(tc.tile_pool(name="z", bufs=1))

        t2 = cpool.tile([NROW, NSH], DT.float16)
        si = cpool.tile([NROW, KPAD], DT.float16)
        qh = cpool.tile([P, NSH], DT.float16)
        ah = cpool.tile([P, AWK], DT.float16)
        al = cpool.tile([P, AWK], DT.float16)
        vprT = cpool.tile([P, KCH * D], DT.bfloat16)
        vpiT = cpool.tile([P, KCH * D], DT.bfloat16)
        ident = cpool.tile([P, P], DT.bfloat16)
        nc.sync.dma_start(qh[:], qh_d)
        nc.sync.dma_start(ah[:], ah_d)
        nc.sync.dma_start(al[:], al_d)
        nc.sync.dma_start(si[:], si_d)
        nc.sync.dma_start(t2[:], t2_d)
        nc.sync.dma_start(vprT[:], vprT_d)
        nc.sync.dma_start(vpiT[:], vpiT_d)
        nc.sync.dma_start(ident[:], ident_d)

        sm = smpool.tile([P, NC2 * 4096], DT.bfloat16)
        zacc = zpool.tile([P, 4 * NC2], DT.float32)

        # ---- pass 1: aw (2-term fp16) -> softmax (4 quarters per n-chunk)
        with tc.tile_pool(name="awps", bufs=1, space="PSUM") as awps:
            for c8 in range(NC2):
                nsl = slice(c8 * P, (c8 + 1) * P)
                awq = []
                mxs = []
                for q in range(4):
                    aw = awps.tile([P, 1024], DT.float32, tag=f"aw{q}")
                    for j in range(2):
                        ksl = slice(q * 1024 + j * 512, q * 1024 + (j + 1) * 512)
                        osl = slice(j * 512, (j + 1) * 512)
                        nc.tensor.matmul(aw[:, osl], qh[:, nsl], ah[:, ksl],
                                         start=True, stop=False)
                        nc.tensor.matmul(aw[:, osl], qh[:, nsl], al[:, ksl],
                                         start=False, stop=True)
                    mx = wpool.tile([P, 1], DT.float32, tag=f"mx{q}")
                    nc.vector.reduce_max(mx[:], aw[:], axis=AX.X)
                    awq.append(aw)
                    mxs.append(mx)
                mxa = wpool.tile([P, 1], DT.float32, tag="mxa")
                mxb = wpool.tile([P, 1], DT.float32, tag="mxb")
                nc.vector.tensor_tensor(mxa[:], mxs[0][:], mxs[1][:], ALU.max)
                nc.vector.tensor_tensor(mxb[:], mxs[2][:], mxs[3][:], ALU.max)
                mxc = wpool.tile([P, 1], DT.float32, tag="mxc")
                nc.vector.tensor_tensor(mxc[:], mxa[:], mxb[:], ALU.max)
                negmx = wpool.tile([P, 1], DT.float32, tag="negmx")
                nc.vector.tensor_scalar_mul(negmx[:], mxc[:], -1.0)
                for q in range(4):
                    nc.scalar.activation(
                        sm[:, c8 * AWK + q * 1024: c8 * AWK + (q + 1) * 1024],
                        awq[q][:], F.Exp, bias=negmx[:],
                        accum_out=zacc[:, 4 * c8 + q: 4 * c8 + q + 1])

        # ---- pass 2: phases -> sin/cos -> PE-transpose sm -> inverse GEMM
        with (tc.tile_pool(name="php", bufs=4, space="PSUM") as php,
              tc.tile_pool(name="smtp", bufs=2, space="PSUM") as smtp,
              tc.tile_pool(name="ops", bufs=1, space="PSUM") as ops):
            outT = ops.tile([P, NSH], DT.float32)
            phs = {}

            def emit_ph(j):          # j = half index 0..59 (2 per k-chunk)
                if j >= 2 * KCH:
                    return
                t = php.tile([P, 512], DT.float32, tag="ph")
                nc.tensor.matmul(t[:], si[:, (j // 2) * P:(j // 2 + 1) * P],
                                 t2[:, (j % 2) * 512:(j % 2 + 1) * 512],
                                 start=True, stop=True)
                phs[j] = t

            def emit_inv(kc, smC, smS):
                dsl = slice(kc * D, (kc + 1) * D)
                for h in range(2):
                    sl = slice(h * 512, (h + 1) * 512)
                    nc.tensor.matmul(outT[:, sl], vprT[:, dsl], smC[:, sl],
                                     start=(kc == 0), stop=False)
                    nc.tensor.matmul(outT[:, sl], vpiT[:, dsl], smS[:, sl],
                                     start=False, stop=(kc == KCH - 1))

            for j in range(4):
                emit_ph(j)
            prev = None
            for kc in range(KCH):
                r = wpool.tile([P, 1024], DT.float32, tag="r")
                w2 = wpool.tile([P, 1024], DT.float32, tag="w2")
                for h in range(2):
                    j = 2 * kc + h
                    sl = slice(h * 512, (h + 1) * 512)
                    nc.vector.add_range_wrap(r[:, sl], phs[j][:], 0.0, 0.5, 1.0)
                    nc.vector.add_range_wrap(w2[:, sl], r[:, sl], 0.25, 0.5, 1.0)
                    del phs[j]
                emit_ph(2 * kc + 4)
                emit_ph(2 * kc + 5)
                smT = smtp.tile([P, 1024], DT.bfloat16, tag="smT")
                for c8 in range(NC2):
                    nc.tensor.transpose(
                        smT[:, c8 * P:(c8 + 1) * P],
                        sm[:, c8 * AWK + kc * P: c8 * AWK + (kc + 1) * P],
                        ident[:])
                sint = wpool.tile([P, 1024], DT.bfloat16, tag="sint")
                cost = wpool.tile([P, 1024], DT.bfloat16, tag="cost")
                nc.scalar.activation(cost[:], w2[:], F.Sin, scale=TWOPI)
                nc.scalar.activation(sint[:], r[:], F.Sin, scale=TWOPI)
                smC = wpool.tile([P, 1024], DT.bfloat16, tag="smC")
                smS = wpool.tile([P, 1024], DT.bfloat16, tag="smS")
                nc.vector.tensor_mul(smC[:], smT[:], cost[:])
                nc.vector.tensor_mul(smS[:], smT[:], sint[:])
                if prev is not None:
                    emit_inv(kc - 1, *prev)
                prev = (smC, smS)
            emit_inv(KCH - 1, *prev)

            res = wpool.tile([P, NSH], DT.float32, tag="res")
            nc.vector.tensor_copy(res[:], outT[:])
            nc.sync.dma_start(outT_d, res[:])
            nc.sync.dma_start(zs_d, zacc[:])

    nc.compile()
    return nc


# ---------------------------------------------------------------- profiling
def enable_ntff_profiling():
    """Provide the antenv.axon_hooks module run_bass_kernel_spmd needs for
    trace=True under axon, backed by trn_boot's ctypes NTFF hook."""
    import types
    if "antenv.axon_hooks" in sys.modules:
        return True
    sys.path.insert(0, "/root/.axon_site")
    try:
        from trn_agent_boot.trn_boot import _ntff_profile_via_ctypes
        hook = _ntff_profile_via_ctypes("/opt/axon/libaxon_pjrt.so")
    except Exception as e:
        print(f"ntff hook unavailable: {e}")
        return False
    if hook is None:
        print("ntff hook: .so lacks axon_start_nrt_profile")
        return False
    mod = types.ModuleType("antenv.axon_hooks")
    mod._hook = hook
    mod.get_axon_ntff_profile_hook = lambda: mod._hook
    mod.set_axon_ntff_profile_hook = lambda h: setattr(mod, "_hook", h)
    sys.modules["antenv.axon_hooks"] = mod
    # upload_artifacts copies the NEFF dir to a remote bucket -- hangs in
    # this container; keep artifacts local instead.
    import concourse.bass_utils as bu
    bu.upload_artifacts = lambda tmpdir: tmpdir
    return True


# ---------------------------------------------------------------- runner
_NC1 = None
_NC2 = None


def run_ewald(q_vector, k_vector, v_vector, positions, cell, batch, k_fwd,
              k_inv, trace=False):
    global _NC1, _NC2
    if trace:
        trace = enable_ntff_profiling()
    th, tl, sf, si, kvh, kvl, vvh, qh, ql = host_prep(
        q_vector, k_vector, v_vector, positions, cell, k_fwd, k_inv)

    kvh_c = chunk_major(kvh)
    vvh_c = chunk_major(vvh)

    if _NC1 is None:
        _NC1 = build_k1()
    in1 = [{"th": th, "tl": tl,
            "sf": np.ascontiguousarray(sf[:, c * KSH:(c + 1) * KSH]),
            "kvh": kvh_c, "vvh": vvh_c} for c in range(8)]
    r1 = run_bass_kernel_spmd(_NC1, in1, list(range(8)), trace=trace)

    K = k_fwd.shape[0]
    kre = np.concatenate([r1.results[c]["kre"] for c in range(8)], axis=1)
    kim = np.concatenate([r1.results[c]["kim"] for c in range(8)], axis=1)
    vre = np.concatenate(
        [r1.results[c]["vre"].astype(np.float32) for c in range(8)], axis=1)
    vim = np.concatenate(
        [r1.results[c]["vim"].astype(np.float32) for c in range(8)], axis=1)
    akp = np.zeros((D, AWK), dtype=np.float32)
    akp[:, :KPAD] = np.hypot(kre, kim)
    akp[:, K:] = 0.0
    ah, al = split16(akp)
    vprT = chunk_major(np.ascontiguousarray(vre.T).astype(bf16))  # [P, KCH*D]
    vpiT = chunk_major(np.ascontiguousarray(vim.T).astype(bf16))
    ident = np.eye(P, dtype=np.float32).astype(bf16)

    if _NC2 is None:
        _NC2 = build_k2()
    in2 = [{"t2": np.ascontiguousarray(th[:, c * NSH:(c + 1) * NSH]),
            "si": si,
            "qh": np.ascontiguousarray(qh[:, c * NSH:(c + 1) * NSH]),
            "ah": ah, "al": al, "vprT": vprT, "vpiT": vpiT, "ident": ident}
           for c in range(8)]
    r2 = run_bass_kernel_spmd(_NC2, in2, list(range(8)), trace=trace)

    outs = []
    for c in range(8):
        oT = r2.results[c]["outT"]                    # [128 d, 1024 n]
        zs = r2.results[c]["zs"]                      # [128, 16]
        z = (zs[:, 0::4] + zs[:, 1::4] + zs[:, 2::4]
             + zs[:, 3::4]).T.reshape(-1)               # atom n=c8*128+p
        outs.append((oT.T / z[:, None]).astype(np.float32))
    out = np.concatenate(outs, axis=0)
    return out, (r1, r2)


# ---------------------------------------------------------------- entry point
def kernel(q_vector, k_vector, v_vector, positions, cell, batch, k_fwd, k_inv):
    """Full-input entry: shards across 8 NeuronCores internally."""
    out, _ = run_ewald(np.asarray(q_vector), np.asarray(k_vector),
                       np.asarray(v_vector), np.asarray(positions),
                       np.asarray(cell), np.asarray(batch),
                       np.asarray(k_fwd), np.asarray(k_inv))
    return out
